# revision 1
# baseline (speedup 1.0000x reference)
"""Trainium2 Bass kernel for batched 22-node complete-digraph GNN.

Model (per reference):
    x0 = relu(features @ W_lift + b_lift)            # [N, 15]
    agg1 = segment_sum(x0[src], dst)                 # complete digraph w/ self
    x1 = relu(agg1 @ W1 + b1)
    agg2 = segment_sum(x1[src], dst)
    x2 = relu(agg2 @ W2 + b2)
    out = x2.reshape(B, 110) @ W_ro + b_ro           # [B, 1]

Each 22-node graph is a complete digraph with self-loops, so the edge
aggregation is "every node receives the sum over all 22 nodes of its graph":
    S1_g   = sum_{i in graph g} relu(lift(f_i))      # [B, 15]
    h1_g   = relu(S1_g @ W1 + b1)                    # same for all nodes of g
    x2_g   = relu((22*h1_g) @ W2 + b2)               # [B, 5]
    out_g  = x2_g @ (W_ro.reshape(22,5).sum(0)) + b_ro

Sharding: data-parallel over graphs, B=32768 split across 8 cores
(4096 graphs / 90112 nodes per core).  No cross-core communication.

Per-core device schedule:
  - nodes in 8 blocks of 11264 (=512 graphs * 22); block a on partitions
    [9a, 9a+9) feature-major (host pre-transposed, fp16).
  - lift: block-diagonal lhsT [72, 120] (8 x W_lift) -> PSUM [120, cols],
    8 nodes per moving column.
  - evac: bias+relu fused, on ScalarE (activation) or VectorE (tensor_scalar)
    per chunk (tunable split) -> x0 [120, 1408] f32.
  - graph sum: VectorE tensor_reduce (3D AP, innermost 22) -> S1 fp16;
    optionally GPSIMD pre-adds node pairs (22->11) for some chunks.
  - per-graph MLP: 3 tiny block-diagonal matmuls + activations, emitted
    after all lift matmuls (so its chains never block the PE FIFO), in
    256-graph segments.
"""

import os

import numpy as np

B = 32768
G = 22
N = B * G
NCORES = 8
BC = B // NCORES          # 4096 graphs per core
NC_NODES = BC * G         # 90112 nodes per core
NBLK = 8                  # node blocks per core (partition blocks)
CHUNK_NODES = NC_NODES // NBLK   # 11264 = 512 graphs * 22
CHUNK_GRAPHS = BC // NBLK        # 512
FT_P = 9 * NBLK           # 72 partitions for features
LIFT_P = 15 * NBLK        # 120 partitions for x0 / S1

USE_FP16 = os.environ.get("KERNEL_FP32", "0") != "1"

# node-column chunk sizes (multiples of 22, each <= 1408 = 3 psum banks);
# small chunks at both ends: fast pipeline fill and a short tail
CHUNKS = [int(c) for c in os.environ.get(
    "KERNEL_CHUNKS",
    "704,1408,1408,1408,1408,1408,1408,1408,704").split(",")]
assert sum(CHUNKS) == CHUNK_NODES and all(c % G == 0 for c in CHUNKS)
NCHUNK = len(CHUNKS)
CUMG = []  # cumulative graph counts after each chunk
_g = 0
for _c in CHUNKS:
    _g += _c // G
    CUMG.append(_g)

# tuning knobs (sweepable): per-chunk evac engine (A=ScalarE, D=VectorE) and
# whether GPSIMD pre-reduces node pairs 22->11 (P) or not (N)
EVAC_CFG = os.environ.get("KERNEL_EVAC",
                          "DAAAADAAA" if NCHUNK == 9 else "A" * NCHUNK)
POOL_CFG = os.environ.get("KERNEL_POOL",
                          "NNNPPPNPP" if NCHUNK == 9 else "N" * NCHUNK)
SPLIT_F = float(os.environ.get("KERNEL_SPLITF", "0.75"))

WPACK_COLS = 256          # last 8 fp16 cols hold the 4 fp32 biases (bitcast)

LAST_RESULT = None


def _structured(src, dst):
    offsets = np.repeat(np.arange(B, dtype=np.int32) * G, G * G)
    ls = np.tile(np.repeat(np.arange(G, dtype=np.int32), G), B)
    ld = np.tile(np.tile(np.arange(G, dtype=np.int32), G), B)
    return np.array_equal(src, offsets + ls) and np.array_equal(dst, offsets + ld)


def _fallback_numpy(features, src, dst, W_lift, b_lift, W1, b1, W2, b2, W_ro, b_ro):
    x = np.maximum(features @ W_lift + b_lift, 0.0)
    agg = np.zeros((N, x.shape[1]), np.float32)
    np.add.at(agg, dst, x[src])
    x = np.maximum(agg @ W1 + b1, 0.0)
    agg = np.zeros((N, x.shape[1]), np.float32)
    np.add.at(agg, dst, x[src])
    x = np.maximum(agg @ W2 + b2, 0.0)
    x = x.reshape(B, G * 5)
    return (x @ W_ro + b_ro).astype(np.float32)


def _block_diag(W, nblk, dtype):
    fi, fo = W.shape
    out = np.zeros((fi * nblk, fo * nblk), dtype)
    for a in range(nblk):
        out[fi * a:fi * (a + 1), fo * a:fo * (a + 1)] = W
    return out


_cached = {}


def _build_kernel():
    import concourse.bacc as bacc
    import concourse.mybir as mybir
    from concourse.tile import TileContext

    fdt = mybir.dt.float16 if USE_FP16 else mybir.dt.float32
    f32 = mybir.dt.float32
    Relu = mybir.ActivationFunctionType.Relu
    Ident = mybir.ActivationFunctionType.Identity

    # Bacc (not raw Bass): its compile() splits multi-wait sync conditions
    # into event semaphores, which TRN2 engine instructions require (1 wait).
    nc = bacc.Bacc(trn_type="TRN2", enable_partition_id=False)

    # head = wpack (weights + bitcast biases) and ft chunk 0 merged into one
    # tensor so the whole prologue is a single DMA descriptor set
    head_cols = WPACK_COLS + CHUNKS[0]
    head_d = nc.dram_tensor("head", [LIFT_P, head_cols], fdt,
                            kind="ExternalInput")
    ft_d = nc.dram_tensor("ft", [FT_P, CHUNK_NODES - CHUNKS[0]], fdt,
                          kind="ExternalInput")
    out_d = nc.dram_tensor("out", [NBLK, CHUNK_GRAPHS], f32,
                           kind="ExternalOutput")

    act_set = {t for t in range(NCHUNK) if EVAC_CFG[t] == "A"}
    pool_set = {t for t in range(NCHUNK) if POOL_CFG[t] == "P"}

    with TileContext(nc) as tc:
        with (
            tc.tile_pool(name="consts", bufs=1) as consts,
            tc.tile_pool(name="ft", bufs=1) as ftp,
            tc.tile_pool(name="x0", bufs=1) as x0p,
            tc.tile_pool(name="tmp", bufs=1) as tmpp,
            tc.tile_pool(name="s1", bufs=1) as s1p,
            tc.tile_pool(name="ps", bufs=2, space="PSUM") as psp,
        ):
            starts = [sum(CHUNKS[:i]) for i in range(NCHUNK)]

            # head (weights/biases + ft chunk 0) first so compute starts
            # ASAP, then the remaining ft chunks.  The SP DMA queue is FIFO:
            # nothing with a sync-wait may sit in front of a load.
            altq = os.environ.get("KERNEL_ALTQ", "0") == "1"
            head_sb = consts.tile([LIFT_P, WPACK_COLS + CHUNKS[0]], fdt)
            nc.sync.dma_start(out=head_sb, in_=head_d[:, :])
            wpack_sb = head_sb[:, 0:WPACK_COLS]
            ft_tiles = [None] * NCHUNK
            ft_tiles[0] = head_sb[0:FT_P, WPACK_COLS:WPACK_COLS + CHUNKS[0]]
            for d in range(1, NCHUNK):
                ft_tiles[d] = ftp.tile([FT_P, CHUNKS[d]], fdt, tag=f"ft{d}",
                                       name=f"ft_sb{d}")
                # alternate the two HWDGE rings (SP / ACT sequencer) so
                # descriptor generation pipelines in parallel
                eng = nc.scalar if (altq and d % 2 == 0) else nc.sync
                eng.dma_start(
                    out=ft_tiles[d],
                    in_=ft_d[:, starts[d] - CHUNKS[0]:
                             starts[d] - CHUNKS[0] + CHUNKS[d]])

            # PE p-state warm-up: garbage matmuls on a zeroed tile keep the
            # PE busy from t~0 so the first real lift matmuls run ramped-up
            warm_sb = consts.tile([FT_P, 128], fdt)
            nc.gpsimd.memset(warm_sb, 0.0)
            warm_ps = psp.tile([LIFT_P, 128], f32, tag="warm", bufs=1)
            for _ in range(6):
                nc.tensor.matmul(warm_ps[:, :], lhsT=warm_sb[:, 0:LIFT_P],
                                 rhs=warm_sb[:, :], start=True, stop=True)

            wlift_sb = wpack_sb[0:FT_P, 0:LIFT_P]
            w1_sb = wpack_sb[:, 120:200]
            w2_sb = wpack_sb[0:10 * NBLK, 200:240]
            wro_sb = wpack_sb[0:5 * NBLK, 240:248]
            bias_f32 = wpack_sb[:, 248:256].bitcast(f32)   # [120, 4]
            blift = bias_f32[:, 0:1]
            b1 = bias_f32[0:10 * NBLK, 1:2]
            b2 = bias_f32[0:5 * NBLK, 2:3]
            bro = bias_f32[0:NBLK, 3:4]

            # absorb the const-DMA wait once per consumer engine
            dummy = consts.tile([LIFT_P, 4], f32)
            nc.scalar.copy(out=dummy, in_=bias_f32[:, 0:4])
            dummy2 = consts.tile([LIFT_P, 1], f32)
            nc.vector.tensor_copy(out=dummy2, in_=bias_f32[:, 0:1])

            s1_sb = s1p.tile([LIFT_P, CHUNK_GRAPHS], fdt)

            def s2act(eng, out, in_, bias, relu):
                if eng == "A":
                    nc.scalar.activation(out=out, in_=in_,
                                         func=Relu if relu else Ident,
                                         bias=bias, scale=1.0)
                elif relu:
                    nc.vector.tensor_scalar(
                        out=out, in0=in_, scalar1=bias, scalar2=0.0,
                        op0=mybir.AluOpType.add, op1=mybir.AluOpType.max)
                else:
                    nc.vector.tensor_scalar(
                        out=out, in0=in_, scalar1=bias, scalar2=None,
                        op0=mybir.AluOpType.add)

            def stage2(lo, hi, eng):
                """per-graph MLP on graph columns [lo, hi)."""
                w = hi - lo
                cs = slice(lo, hi)
                h1_ps = psp.tile([10 * NBLK, 512], f32, tag="s2", bufs=1,
                                 name=f"h1ps_{lo}")
                nc.tensor.matmul(h1_ps[:, 0:w], lhsT=w1_sb, rhs=s1_sb[:, cs],
                                 start=True, stop=True)
                h1_sb = s1p.tile([10 * NBLK, 512], fdt, tag=f"h1_{lo}",
                                 name=f"h1sb_{lo}")
                s2act(eng, h1_sb[:, 0:w], h1_ps[:, 0:w], b1, True)
                h2_ps = psp.tile([5 * NBLK, 512], f32, tag="s2", bufs=1,
                                 name=f"h2ps_{lo}")
                nc.tensor.matmul(h2_ps[:, 0:w], lhsT=w2_sb, rhs=h1_sb[:, 0:w],
                                 start=True, stop=True)
                h2_sb = s1p.tile([5 * NBLK, 512], fdt, tag=f"h2_{lo}",
                                 name=f"h2sb_{lo}")
                s2act(eng, h2_sb[:, 0:w], h2_ps[:, 0:w], b2, True)
                o_ps = psp.tile([NBLK, 512], f32, tag="s2", bufs=1, name=f"ops_{lo}")
                nc.tensor.matmul(o_ps[:, 0:w], lhsT=wro_sb, rhs=h2_sb[:, 0:w],
                                 start=True, stop=True)
                o_sb = s1p.tile([NBLK, 512], f32, tag=f"o_{lo}",
                                name=f"osb_{lo}")
                s2act(eng, o_sb[:, 0:w], o_ps[:, 0:w], bro, False)
                nc.sync.dma_start(out=out_d[:, cs], in_=o_sb[:, 0:w])

            def chunk_matmuls(t):
                ft_sb = ft_tiles[t]
                w = CHUNKS[t]
                if fdt != f32:
                    # absorb the ft-DMA wait on PE so the matmuls only wait
                    # on the psum-slot release
                    nc.tensor.ldweights(weights=ft_sb[:, 0:128])
                ps = psp.tile([LIFT_P, 1408], f32, tag="ps", name=f"ps_{t}")
                col = 0
                while col < w:
                    n = min(512, w - col)
                    nc.tensor.matmul(
                        ps[:, col:col + n],
                        lhsT=wlift_sb,
                        rhs=ft_sb[:, col:col + n],
                        start=True, stop=True,
                    )
                    col += n
                return ps

            def chunk_evac_reduce(t, ps):
                w = CHUNKS[t]
                ng = w // G
                # fused bias+relu evacuation PSUM -> SBUF
                x0_sb = x0p.tile([LIFT_P, w], mybir.dt.float32, tag=f"x0_{t}",
                                 name=f"x0_{t}")
                if EVAC_CFG[t] == "S":
                    # split the evacuation: ScalarE front, VectorE back
                    sp = (int(w * SPLIT_F) // 64) * 64
                    nc.scalar.activation(out=x0_sb[:, 0:sp], in_=ps[:, 0:sp],
                                         func=Relu, bias=blift, scale=1.0)
                    nc.vector.tensor_scalar(
                        out=x0_sb[:, sp:w], in0=ps[:, sp:w],
                        scalar1=blift, scalar2=0.0,
                        op0=mybir.AluOpType.add, op1=mybir.AluOpType.max)
                elif t in act_set:
                    nc.scalar.activation(out=x0_sb[:, :], in_=ps[:, 0:w],
                                         func=Relu, bias=blift, scale=1.0)
                else:
                    nc.vector.tensor_scalar(
                        out=x0_sb[:, :], in0=ps[:, 0:w],
                        scalar1=blift, scalar2=0.0,
                        op0=mybir.AluOpType.add, op1=mybir.AluOpType.max)
                g0 = CUMG[t] - ng
                sl = s1_sb[:, g0:CUMG[t]]
                with nc.allow_low_precision(reason="fp16 S1 is plenty"):
                    if t in pool_set:
                        # GPSIMD pre-adds node pairs: 22 -> 11
                        tmp = tmpp.tile([LIFT_P, ng, 11], mybir.dt.float32,
                                        tag=f"tmp_{t}", name=f"tmp_{t}")
                        x0_4d = x0_sb.rearrange("p (g r two) -> p g r two",
                                                two=2, r=11)
                        nc.gpsimd.tensor_add(tmp[:, :, :],
                                             x0_4d[:, :, :, 0],
                                             x0_4d[:, :, :, 1])
                        nc.vector.tensor_reduce(
                            out=sl, in_=tmp[:, :, :],
                            axis=mybir.AxisListType.X,
                            op=mybir.AluOpType.add)
                    else:
                        nc.vector.tensor_reduce(
                            out=sl, in_=x0_sb.rearrange("p (g i) -> p g i",
                                                        i=G),
                            axis=mybir.AxisListType.X,
                            op=mybir.AluOpType.add)

            # stage-2 segments: (graph_lo, graph_hi, placed_after_chunk, eng).
            # Placement keeps each segment's deps >=2 chunks old so its
            # matmuls never block the lift pipeline in the PE FIFO; tail
            # segments use ScalarE (free after the last evac).
            seg_bounds = [int(x) for x in os.environ.get(
                "KERNEL_SEGS", "0,352,512").split(",")]
            s2eng = os.environ.get("KERNEL_S2ENG", "A")
            for t in range(NCHUNK):
                ps = chunk_matmuls(t)
                chunk_evac_reduce(t, ps)
            # stage-2 only after every lift matmul is emitted, so its chains
            # never block the lift pipeline in the PE FIFO
            for i in range(len(seg_bounds) - 1):
                stage2(seg_bounds[i], seg_bounds[i + 1], s2eng)

    if not nc.is_finalized():
        nc.finalize()
    return nc


def kernel(features, src, dst, W_lift, b_lift, W1, b1, W2, b2, W_ro, b_ro):
    global LAST_RESULT
    features = np.asarray(features, np.float32)
    src = np.asarray(src, np.int32)
    dst = np.asarray(dst, np.int32)
    W_lift = np.asarray(W_lift, np.float32)
    b_lift = np.asarray(b_lift, np.float32)
    W1 = np.asarray(W1, np.float32)
    b1 = np.asarray(b1, np.float32)
    W2 = np.asarray(W2, np.float32)
    b2 = np.asarray(b2, np.float32)
    W_ro = np.asarray(W_ro, np.float32)
    b_ro = np.asarray(b_ro, np.float32)

    if not _structured(src, dst):
        return _fallback_numpy(features, src, dst, W_lift, b_lift,
                               W1, b1, W2, b2, W_ro, b_ro)

    npdt = np.float16 if USE_FP16 else np.float32

    # features -> per-core feature-major block layout [NCORES, 72, 11264]
    ft = (features.reshape(NCORES, NBLK, CHUNK_NODES, 9)
          .transpose(0, 1, 3, 2)
          .reshape(NCORES, FT_P, CHUNK_NODES)
          .astype(npdt))

    wpack = np.zeros((LIFT_P, WPACK_COLS), npdt)
    wpack[0:FT_P, 0:LIFT_P] = _block_diag(W_lift, NBLK, npdt)
    wpack[0:LIFT_P, 120:200] = _block_diag(W1, NBLK, npdt)
    wpack[0:10 * NBLK, 200:240] = _block_diag((G * W2).astype(np.float32),
                                              NBLK, npdt)
    wro_eff = W_ro.reshape(G, 5).sum(axis=0)
    for a in range(NBLK):
        wpack[5 * a:5 * (a + 1), 240 + a] = wro_eff

    bpack = np.zeros((LIFT_P, 4), np.float32)
    bpack[:, 0] = np.tile(b_lift, NBLK)
    bpack[0:10 * NBLK, 1] = np.tile(b1, NBLK)
    bpack[0:5 * NBLK, 2] = np.tile(b2, NBLK)
    bpack[0:NBLK, 3] = float(b_ro[0])
    # bit-pack the fp32 biases into the trailing fp16 columns (bitcast)
    if USE_FP16:
        wpack[:, 248:256] = bpack.view(np.float16)
    else:
        wpack[:, 248:252] = bpack

    if "nc" not in _cached:
        _cached["nc"] = _build_kernel()
    nc = _cached["nc"]

    from concourse import bass_utils

    in_maps = []
    for c in range(NCORES):
        head = np.zeros((LIFT_P, WPACK_COLS + CHUNKS[0]), npdt)
        head[:, 0:WPACK_COLS] = wpack
        head[0:FT_P, WPACK_COLS:] = ft[c, :, 0:CHUNKS[0]]
        in_maps.append({
            "head": head,
            "ft": np.ascontiguousarray(ft[c, :, CHUNKS[0]:]),
        })

    trace = os.environ.get("KERNEL_TRACE", "0") == "1"
    res = None
    for attempt in range(4):
        try:
            res = bass_utils.run_bass_kernel_spmd(
                nc, in_maps, core_ids=list(range(NCORES)), trace=trace,
            )
            break
        except ModuleNotFoundError:
            # axon client without the NTFF profile hook: retry untraced
            trace = False
        except Exception as e:  # noqa: BLE001
            # transient NRT_EXEC_UNIT_UNRECOVERABLE flakes recover on retry
            if attempt == 3 or "UNRECOVERABLE" not in str(e).upper():
                raise
            import time
            time.sleep(15)
    LAST_RESULT = res

    out = np.concatenate([r["out"].reshape(-1) for r in res.results])
    return np.ascontiguousarray(out.reshape(B, 1).astype(np.float32))

